# revision 37
# baseline (speedup 1.0000x reference)
"""Locally-connected Conv2d (unique weights per output location) on 8 trn2 cores.

Problem (hardcoded): x [256,1,280,280] f32, weight [12800,1,28,28] f32,
bias [12800,1] f32 -> out [256,128,10,10] f32.  kernel 28x28, stride 28
(non-overlapping patches), 10x10=100 locations, 128 filters.

Per location l the computation is a plain matmul:
    out[b, f, l] = sum_k patch[b, l, k] * w[f, l, k] + bias[f, l],  k in [0,784)

Strategy: shard the 100 locations across 8 cores (pad to 104 = 8*13).
Host-side we repack x into k-major patch layout and weights into k-major
filter layout (both fp16 to halve HBM traffic; accumulation is fp32 in
PSUM), so the device does nothing but streaming matmuls:
    per location: 7 accumulating matmuls [112k x 128f]^T @ [112k x 256b]
    + one K=1 matmul (ones x bias row) that folds in the bias.

Environment-driven constraints (this walrus build / axon runtime):
  - each DMA / matmul / ldweights / Pool-copy instruction may carry at most
    ONE sync-wait command; Tile adds a lane-reuse wait to any DMA past the
    8th on a lane group, so data-dependent stores launder their wait
    through a Pool carrier op (per-engine wait elision).
  - HWDGE (nc.sync) feeds a single SDMA engine here (~27 GB/s); SWDGE
    (nc.gpsimd) sprays better -> all DMA goes through gpsimd.
"""

import numpy as np

import concourse.bass as bass
import concourse.mybir as mybir
from concourse import bass_utils
from concourse.tile import TileContext
from concourse.vector_clock import ScopedClock


def _split_drain_and_barrier(self, tick_clock, wait_clock):
    """TileContext._drain_and_barrier with the tail drain's sem waits split
    across several drain instructions: this walrus build caps the number of
    sync-wait commands a single instruction may carry."""
    drain_inst = self.nc.sync.drain()
    wait_clock.add_sem_waits(
        drain_inst.ins, ScopedClock({None: tick_clock.global_clock}))
    mi = drain_inst.ins
    if mi.sync_info is not None and mi.sync_info.on_wait:
        waits = list(mi.sync_info.on_wait)
        ups = list(mi.sync_info.on_update or [])
        mi.sync_info = mybir.SyncInfo(on_wait=waits[:1], on_update=ups)
        for w in waits[1:]:
            extra = self.nc.sync.drain()
            extra.ins.sync_info = mybir.SyncInfo(on_wait=[w], on_update=[])
    self.nc.all_engine_barrier(sem_only=True)
    assert self.sems is not None
    popped = self.nc._tile_sem_poison_stack.pop()
    assert popped is self._sem_poison
    if not SKIP_TAIL_CLEAR:
        self.nc.clear_and_free_semaphores(list(self.sems.allocated().values()))
        self.nc.all_engine_barrier(sem_only=True)


SKIP_TAIL_CLEAR = True

TileContext._drain_and_barrier = _split_drain_and_barrier

B = 256       # batch
NF = 128      # filters
HS = WS = 10  # output spatial
L = HS * WS   # locations
KH = KW = 28  # kernel == stride (non-overlapping)
K = KH * KW   # contraction length per location (784)
NCORES = 8
LPC = 13      # locations per core (8*13 = 104 >= 100, tail zero-padded)
LPAD = NCORES * LPC
KC = 7        # contraction chunks
KP = 112      # partitions per chunk (7*112 = 784); kh splits as (7,4)

# location blocks per core: pair-aligned (pairs share a PSUM bank)
BLOCKS = [(0, 5), (5, 11), (11, 13)]
STORE_BLOCKS = [(0, 8), (8, 13)]

_CACHED = {}


def _build_bass():
    nc = bass.Bass(trn_type="TRN2")
    xks = [nc.dram_tensor(f"xk{i}", [KP, l1 - l0, KC, B], mybir.dt.float16,
                          kind="ExternalInput")
           for i, (l0, l1) in enumerate(BLOCKS)]
    wks = [nc.dram_tensor(f"wk{i}", [KP, l1 - l0, KC, NF], mybir.dt.float16,
                          kind="ExternalInput")
           for i, (l0, l1) in enumerate(BLOCKS)]
    bk = nc.dram_tensor("bk", [1, LPC, NF], mybir.dt.float16,
                        kind="ExternalInput")
    # separate store tensors: avoids per-tensor WAW chaining between stores
    outs = [nc.dram_tensor(f"out{i}", [NF, l1 - l0, B], mybir.dt.float16,
                           kind="ExternalOutput")
            for i, (l0, l1) in enumerate(STORE_BLOCKS)]

    NPAIR = (LPC + 1) // 2

    with TileContext(nc) as tc:
        with (
            tc.tile_pool(name="xp", bufs=len(BLOCKS)) as xpool,
            tc.tile_pool(name="wp", bufs=len(BLOCKS)) as wpool,
            tc.tile_pool(name="bp", bufs=1) as bpool,
            tc.tile_pool(name="op", bufs=2) as opool,
            # 2 locations share one PSUM bank: NPAIR=7 tiles <= 8 banks, so
            # banks are never reused and matmuls need no release wait.
            tc.tile_pool(name="ps", bufs=NPAIR, space="PSUM") as pspool,
        ):
            ones_t = bpool.tile([1, B], mybir.dt.float16, tag="ones")
            nc.vector.memset(ones_t[:], 1.0)
            bias_t = bpool.tile([1, LPC, NF], mybir.dt.float16, tag="bias")
            # tiny; goes on the (slow, single-engine) HWDGE path so all 8
            # SWDGE lanes stay reserved for the 6 block loads + 2 stores
            nc.sync.dma_start(bias_t[:], bk[:])

            x_ts, w_ts = {}, {}
            for i, (l0, l1) in enumerate(BLOCKS):
                nl = l1 - l0
                x_t = xpool.tile([KP, nl, KC, B], mybir.dt.float16, tag="x")
                w_t = wpool.tile([KP, nl, KC, NF], mybir.dt.float16, tag="w")
                nc.gpsimd.dma_start(x_t[:], xks[i][:])
                nc.gpsimd.dma_start(w_t[:], wks[i][:])
                for l in range(l0, l1):
                    x_ts[l] = x_t[:, l - l0]
                    w_ts[l] = w_t[:, l - l0]

            for i, (l0, l1) in enumerate(STORE_BLOCKS):
                o_t = opool.tile([NF, l1 - l0, B], mybir.dt.float16, tag="o")
                for p in range(l0 // 2, (l1 + 1) // 2):
                    pl0, pl1 = 2 * p, min(2 * p + 2, LPC)
                    ps = pspool.tile([NF, pl1 - pl0, B], mybir.dt.float32)
                    for j, l in enumerate(range(pl0, pl1)):
                        for c in range(KC):
                            nc.tensor.matmul(ps[:, j, :], w_ts[l][:, c, :],
                                             x_ts[l][:, c, :],
                                             start=(c == 0), stop=False)
                        # bias: rank-1 update  ps[f, b] += bias[f] * 1
                        nc.tensor.matmul(ps[:, j, :], bias_t[:, l, :],
                                         ones_t[:], start=False, stop=True)
                    nc.vector.tensor_copy(o_t[:, pl0 - l0:pl1 - l0, :], ps[:])
                nc.gpsimd.dma_start(outs[i][:], o_t[:])
    return nc


def _pack_inputs(x, weight, bias):
    # x: [B,1,280,280] f32.  rows = i*28 + kh, kh = c*4 + khm; cols = j*28 + kw
    # xk[p, l=(i,j), c, b] fp16 with p = khm*28 + kw
    xh = x.astype(np.float16).reshape(B, HS, KC, 4, WS, KW)
    # (b, i, c, khm, j, kw) -> (khm, kw, i, j, c, b)
    xt = np.ascontiguousarray(xh.transpose(3, 5, 1, 4, 2, 0))
    xkf = np.zeros((KP, LPAD, KC, B), np.float16)
    xkf[:, :L] = xt.reshape(KP, L, KC, B)

    # weight: [NF*L, 1, 28, 28] -> [f, l, c, khm, kw] -> [(khm,kw), l, c, f]
    wh = weight.astype(np.float16).reshape(NF, L, KC, 4, KW)
    wt = np.ascontiguousarray(wh.transpose(3, 4, 1, 2, 0)).reshape(KP, L, KC, NF)
    wkf = np.zeros((KP, LPAD, KC, NF), np.float16)
    wkf[:, :L] = wt

    bkf = np.zeros((1, LPAD, NF), np.float16)
    bkf[0, :L] = bias.astype(np.float16).reshape(NF, L).T

    in_maps = []
    for c in range(NCORES):
        base = c * LPC
        m = {"bk": np.ascontiguousarray(bkf[:, base:base + LPC])}
        for i, (l0, l1) in enumerate(BLOCKS):
            m[f"xk{i}"] = np.ascontiguousarray(xkf[:, base + l0:base + l1])
            m[f"wk{i}"] = np.ascontiguousarray(wkf[:, base + l0:base + l1])
        in_maps.append(m)
    return in_maps


def run(x, weight, bias, **run_kwargs):
    """Build+run; returns (output, BassKernelResults)."""
    if "nc" not in _CACHED:
        _CACHED["nc"] = _build_bass()
    nc = _CACHED["nc"]
    in_maps = _pack_inputs(x, weight, bias)
    res = bass_utils.run_bass_kernel_spmd(
        nc, in_maps, core_ids=list(range(NCORES)), **run_kwargs)
    # per core: out{i} is [NF, nl, B]; concat -> [NF, LPC, B]
    outs = np.stack([
        np.concatenate([r["out0"], r["out1"]], axis=1)
        for r in res.results])                        # [8, NF, LPC, B]
    outs = outs.transpose(0, 2, 1, 3).reshape(LPAD, NF, B)[:L]  # [l, f, b]
    out = np.ascontiguousarray(outs.transpose(2, 1, 0)).reshape(B, NF, HS, WS)
    return out.astype(np.float32), res


def kernel(x, weight, bias):
    out, _ = run(x, weight, bias)
    return out
